# revision 9
# baseline (speedup 1.0000x reference)
"""MixProp GNN message-passing kernel for 8 TRN2 NeuronCores.

Reference computation (per batch element b):
    A_n = row_normalize(A + I)
    H_0 = X;  H_k = beta*X + (1-beta) * A_n @_nodes H_{k-1}   (k=1..3)
    out = W @_channels concat([H_0..H_3]) + bias

Kernel strategy:
  - Data-parallel over batch: B=8 batch elements -> 8 cores, no collectives.
  - Host precomputes G_k = polynomial in A_n s.t. H_k = G_k @ X (node-mixing
    and channel-mixing commute, and the hop recurrence is affine in X).
    This removes the sequential hop chain and the beta*X elementwise adds.
  - On device, for each seq position l (all ops are pointwise in l):
      * 4 column-packed matmuls (tile_position=(0,32j)) per 128-node block
        of the contraction build a PSUM tile H0[(src,ch), v] where src =
        (X, G1X, G2X, G3X): lhsT = X[:, l-slice] (stationary, m=32),
        rhs = I / G_k^T (moving).  The X "transpose" comes for free as an
        identity matmul in column group 0.
      * one k=128 conv matmul pair vs W^T produces out[(vh,o), v-half],
        placed in psum partitions 0:64 / 64:128 so the output store runs
        at full 128-partition DMA width.
  - X is DMA'd as [node, (ch, l)] (l-contiguous runs) and reordered on DVE
    to [node, (l, ch)] so that a 32-column lhsT slice is channel-pure.
"""

import sys

sys.path.insert(0, "/opt/trn_rl_repo")

import numpy as np

import concourse.bass as bass
import concourse.bacc as bacc
import concourse.mybir as mybir
from concourse import tile
from concourse import bass_utils

GDEP = 3
BETA = 0.05
C_IN = 32
C_OUT = 64
N = 512
B = 8
L = 256
NB = N // 128  # node blocks of 128

F32 = mybir.dt.float32


class CFG:
    def __init__(self, L=L, Lc=32, mm_dt=mybir.dt.bfloat16, evac_any=False):
        assert L % Lc == 0
        self.L = L
        self.Lc = Lc
        self.mm_dt = mm_dt
        self.evac_any = evac_any


def body(nc, tc, outs, ins, cfg: CFG):
    """Emit the per-core program. ins/outs are dicts of DRAM APs."""
    X_d = ins["x"]          # [C_IN, N, L] storage dtype = mm_dt bytes-compatible f32
    G_d = ins["gt"]         # [GDEP, N, N]  G_k^T
    I_d = ins["ident"]      # [N, N]
    W_d = ins["wt"]         # [128, C_OUT]  W^T
    b_d = ins["bias2"]      # [128, 1]      bias duplicated for (vh, o) rows
    out_d = outs["out"]     # [C_OUT, N, L] f32

    Lc = cfg.Lc
    mm_dt = cfg.mm_dt
    n_chunks = cfg.L // Lc

    with (
        tc.tile_pool(name="const", bufs=1) as cpool,
        tc.tile_pool(name="xraw", bufs=2) as xraw_pool,
        tc.tile_pool(name="xsb", bufs=2) as xsb_pool,
        tc.tile_pool(name="h0sb", bufs=4) as h0sb_pool,
        tc.tile_pool(name="outsb", bufs=2) as out_pool,
        tc.tile_pool(name="h0ps", bufs=2, space="PSUM") as h0ps_pool,
        tc.tile_pool(name="cvps", bufs=2, space="PSUM") as cvps_pool,
    ):
        # ---- constants (replicated, stay resident) ----
        # Every tile a matmul touches is produced by the DVE so each
        # InstMatmult carries at most ONE sync wait (walrus's fused
        # LDWEIGHTS struct only has a single wait slot).
        def const_via_dve(name, src, dt):
            raw = cpool.tile(list(src.shape), F32, name=f"{name}_raw")
            nc.sync.dma_start(raw[:], src)
            t = cpool.tile(list(src.shape), dt, name=name)
            nc.vector.tensor_copy(out=t[:], in_=raw[:])
            return t

        g_t = []  # g_t[k][wb]: [128, N] moving operand for hop k
        for k in range(GDEP):
            row = []
            for wb in range(NB):
                t = const_via_dve(
                    f"g{k}_{wb}", G_d[k, wb * 128:(wb + 1) * 128, :], mm_dt
                )
                row.append(t)
            g_t.append(row)
        i_t = []
        for wb in range(NB):
            t = const_via_dve(f"i_{wb}", I_d[wb * 128:(wb + 1) * 128, :], mm_dt)
            i_t.append(t)
        w_t = const_via_dve("w_t", W_d[:], mm_dt)
        b_t = cpool.tile([128, 1], F32, name="b_t")
        nc.sync.dma_start(b_t[:], b_d[:])

        for ch in range(n_chunks):
            lsl = slice(ch * Lc, (ch + 1) * Lc)

            # ---- X load + (c,l)->(l,c) free-dim reorder ----
            xsb_tiles = []
            for wb in range(NB):
                xraw = xraw_pool.tile(
                    [128, C_IN * Lc], F32, name="xraw", tag=f"xraw{wb}"
                )
                src = X_d[:, wb * 128:(wb + 1) * 128, lsl].rearrange(
                    "c w l -> w c l"
                )
                nc.sync.dma_start(
                    xraw.rearrange("w (c l) -> w c l", l=Lc), src
                )
                xsb = xsb_pool.tile(
                    [128, Lc * C_IN], mm_dt, name="xsb", tag=f"xsb{wb}"
                )
                nc.vector.tensor_copy(
                    out=xsb.rearrange("w (l c) -> w c l", c=C_IN),
                    in_=xraw.rearrange("w (c l) -> w c l", l=Lc),
                )
                xsb_tiles.append(xsb)

            out_sb = out_pool.tile([128, 256 * Lc], F32, name="out_sb")
            out_v = out_sb.rearrange("p (v l) -> p v l", l=Lc)

            # ---- per-seq-position pipeline ----
            prev = None  # deferred conv so PE never waits on the DVE evac
            for l0 in range(Lc):
                h0p = h0ps_pool.tile([128, N], F32, name="h0p")
                xl = [
                    xsb_tiles[wb][:, l0 * C_IN:(l0 + 1) * C_IN]
                    for wb in range(NB)
                ]
                for wb in range(NB):
                    st = wb == 0
                    sp = wb == NB - 1
                    nc.tensor.matmul(
                        h0p[0:32, :], lhsT=xl[wb], rhs=i_t[wb][:],
                        start=st, stop=sp, tile_position=(0, 0),
                        skip_group_check=True,
                    )
                    for k in range(GDEP):
                        j = k + 1
                        nc.tensor.matmul(
                            h0p[32 * j:32 * (j + 1), :], lhsT=xl[wb],
                            rhs=g_t[k][wb][:],
                            start=st, stop=sp, tile_position=(0, 32 * j),
                            skip_group_check=True,
                        )
                h0s = h0sb_pool.tile([128, N], mm_dt, name="h0s")
                if cfg.evac_any:
                    nc.any.tensor_copy(out=h0s[:], in_=h0p[:])
                else:
                    nc.vector.tensor_copy(out=h0s[:], in_=h0p[:])

                if prev is not None:
                    _emit_conv(nc, cvps_pool, w_t, b_t, out_v, *prev)
                prev = (h0s, l0)
            _emit_conv(nc, cvps_pool, w_t, b_t, out_v, *prev)

            # ---- store chunk ----
            for vh in range(2):
                nc.sync.dma_start(
                    out_d[:, vh * 256:(vh + 1) * 256, lsl],
                    out_sb[vh * 64:(vh + 1) * 64, :].rearrange(
                        "o (v l) -> o v l", l=Lc
                    ),
                )


def _emit_conv(nc, cvps_pool, w_t, b_t, out_v, h0s, l0):
    cvp = cvps_pool.tile([128, 256], F32, name="cvp")
    nc.tensor.matmul(
        cvp[0:64, :], lhsT=w_t[:], rhs=h0s[:, 0:256],
        start=True, stop=True, tile_position=(0, 0),
        skip_group_check=True,
    )
    nc.tensor.matmul(
        cvp[64:128, :], lhsT=w_t[:], rhs=h0s[:, 256:512],
        start=True, stop=True, tile_position=(0, 64),
        skip_group_check=True,
    )
    nc.vector.tensor_scalar_add(
        out=out_v[:, :, l0], in0=cvp[:], scalar1=b_t[:, 0:1]
    )


def build_nc(cfg: CFG):
    nc = bacc.Bacc("TRN2", target_bir_lowering=False, debug=False)
    ins = {
        "x": nc.dram_tensor("x", [C_IN, N, cfg.L], F32,
                            kind="ExternalInput").ap(),
        "gt": nc.dram_tensor("gt", [GDEP, N, N], F32,
                             kind="ExternalInput").ap(),
        "ident": nc.dram_tensor("ident", [N, N], F32,
                                kind="ExternalInput").ap(),
        "wt": nc.dram_tensor("wt", [128, C_OUT], F32,
                             kind="ExternalInput").ap(),
        "bias2": nc.dram_tensor("bias2", [128, 1], F32,
                                kind="ExternalInput").ap(),
    }
    outs = {
        "out": nc.dram_tensor("out", [C_OUT, N, cfg.L], F32,
                              kind="ExternalOutput").ap(),
    }
    with tile.TileContext(nc) as tc:
        body(nc, tc, outs, ins, cfg)
    nc.compile()
    return nc


def make_host_inputs(A, W, b):
    """Precompute the replicated operands: G_k^T, I, W^T, bias2."""
    A = np.asarray(A, np.float64)
    n = A.shape[0]
    An = A + np.eye(n)
    An = An / An.sum(axis=1, keepdims=True)
    As = (1.0 - BETA) * An
    eye = np.eye(n)
    G = []
    gk = eye
    for _ in range(GDEP):
        gk = As @ gk + BETA * eye
        G.append(gk)
    GT = np.stack([g.T for g in G]).astype(np.float32)  # [GDEP, N, N]
    ident = np.eye(n, dtype=np.float32)
    WT = np.ascontiguousarray(np.asarray(W, np.float32).T)  # [128, C_OUT]
    b = np.asarray(b, np.float32)
    b2 = np.ascontiguousarray(np.concatenate([b, b]).reshape(128, 1))
    return GT, ident, WT, b2


_NC_CACHE = {}


def run_on_hw(X, A, W, b, cfg=None, trace=False, **spmd_kwargs):
    X = np.ascontiguousarray(np.asarray(X, np.float32))
    GT, ident, WT, b2 = make_host_inputs(A, W, b)
    if cfg is None:
        cfg = CFG()
    key = (cfg.L, cfg.Lc, cfg.mm_dt, cfg.evac_any)
    if key not in _NC_CACHE:
        _NC_CACHE[key] = build_nc(cfg)
    nc = _NC_CACHE[key]
    in_maps = [
        {"x": X[i], "gt": GT, "ident": ident, "wt": WT, "bias2": b2}
        for i in range(B)
    ]
    res = bass_utils.run_bass_kernel_spmd(
        nc, in_maps, core_ids=list(range(B)), trace=trace, **spmd_kwargs
    )
    out = np.stack([res.results[i]["out"] for i in range(B)])
    return out, res


def kernel(X, A, W, b):
    return run_on_hw(X, A, W, b)[0]


if __name__ == "__main__":
    rng = np.random.default_rng(0)
    X = rng.standard_normal((B, C_IN, N, L), dtype=np.float32)
    A = rng.random((N, N), dtype=np.float32)
    W = rng.standard_normal((C_OUT, (GDEP + 1) * C_IN), dtype=np.float32) * 0.1
    b = rng.random(C_OUT, dtype=np.float32)
    out = kernel(X, A, W, b)
    print("out", out.shape, out.dtype, float(np.abs(out).mean()))
